# Initial kernel scaffold
#
"""Bidirectional cross-attention with talking heads — TRN2 Bass kernel.

Sharding: 8 cores = 2 batches x 4 row-blocks of 256 (data parallel over
batch and sequence block). Each core computes, for its (batch, block):
  - out rows  [256, 1024]  (x attends to context, row-softmax path)
  - cout rows [256, 1024]  (context attends to x, col-softmax path)
No collectives; the host assembles the 8 blocks.

Per-core schedule (SBUF-budgeted):
  ctx side: LN -> PE-transpose (feat on partitions) -> cqkT/cqR projection
            -> cv natural projection (kept in SBUF)
  x side:   same -> qkT/qR -> v natural projection (spilled to DRAM)
  path 1 (out rows), reload v, path 2 (cout rows).

Per-path:
  S^T[h]   = matmul(KT_h, QR_h)            [keypos on partitions, qpos free]
  U[h]     = exp(S^T * scale)              (ACT, PSUM->SBUF)
  sigma    = U^T @ ones                    (PE) softmax denominators
  out_h    = U[h].T @ V                    (PE, fp32r, PSUM)
  evict    = (out_h * sigma^-1) * W_th[g,h]  (DVE scalar_tensor_tensor),
             accumulated over h in SBUF  == talking-heads mix fused in
  final    = transpose(acc) @ W_out + b_out  (PE + DVE)

Masks are structurally all-ones for this problem (spec fill=ones); the
reference's mask application is then the identity, so they are ignored.
exp() without max-subtraction is safe: |S*scale| <~ 7.
"""

import os
import numpy as np
from contextlib import ExitStack

_KNOB = lambda k, d: int(os.environ.get(k, d))

P = 128
N_TOK = 1024
DIM = 1024
HEADS = 16
DHEAD = 64
R = 256
SCALE = DHEAD ** -0.5
NCORES = 8

_CACHE = {}


def _patch_tile_drain(tile, mybir):
    """This container's walrus rejects >1 sync wait on an InstDrain
    ("Too many sync wait commands"). Split the TileContext tail drain's
    waits across a chain of single-wait drains on the same engine."""
    if getattr(tile.TileContext, "_drain_split_patched", False):
        return

    def _drain_and_barrier(self, tick_clock, wait_clock):
        drain_inst = self.nc.sync.drain()
        wait_clock.add_sem_waits(
            drain_inst.ins, tile.ScopedClock({None: tick_clock.global_clock})
        )
        si = drain_inst.ins.sync_info
        waits = list(si.on_wait) if si is not None else []
        if len(waits) > 1:
            drain_inst.ins.sync_info = mybir.SyncInfo(
                on_wait=[waits[0]], on_update=list(si.on_update)
            )
            for w in waits[1:]:
                extra = self.nc.sync.drain()
                extra.ins.sync_info = mybir.SyncInfo(on_wait=[w], on_update=[])

        self.nc.all_engine_barrier()
        assert self.sems is not None
        popped = self.nc._tile_sem_poison_stack.pop()
        assert popped is self._sem_poison
        self.nc.clear_and_free_semaphores(list(self.sems.allocated().values()))
        self.nc.all_engine_barrier()

    tile.TileContext._drain_and_barrier = _drain_and_barrier
    tile.TileContext._drain_split_patched = True


_WSPLIT_MAX = 1  # max sync waits this walrus accepts per instruction


def _patch_tile_wait_split(tile, mybir):
    """Split instructions carrying more than _WSPLIT_MAX sem-waits: move the
    excess onto same-engine NoOps committed immediately before (same basic
    block, so engine program order preserves the wait semantics)."""
    if getattr(tile.TileContext, "_wait_split_patched", False):
        return
    orig = tile.TileContext._commit_and_lower
    counter = [0]

    def _commit_and_lower(self, inst, *args, **kwargs):
        si = getattr(inst, "sync_info", None)
        eng = getattr(inst, "engine", None)
        if si is not None and eng is not None and len(si.on_wait) > _WSPLIT_MAX:
            waits = list(si.on_wait)
            keep = waits[-_WSPLIT_MAX:]
            for w in waits[:-_WSPLIT_MAX]:
                counter[0] += 1
                nop = mybir.InstNoOp(
                    name=f"I-wsplit-{counter[0]}",
                    engine=eng, ins=[], outs=[],
                    sync_info=mybir.SyncInfo(on_wait=[w], on_update=[]),
                )
                self._add_instruction(nop)
            inst.sync_info = mybir.SyncInfo(
                on_wait=keep, on_update=list(si.on_update)
            )
        return orig(self, inst, *args, **kwargs)

    tile.TileContext._commit_and_lower = _commit_and_lower
    tile.TileContext._wait_split_patched = True


def build_program():
    import concourse.bass as bass
    import concourse.mybir as mybir
    import concourse.tile as tile
    from concourse.masks import make_identity

    _patch_tile_drain(tile, mybir)
    _patch_tile_wait_split(tile, mybir)

    f32 = mybir.dt.float32
    f32r = mybir.dt.float32r
    ts = bass.ts
    MULT = mybir.AluOpType.mult
    ADD = mybir.AluOpType.add

    nc = bass.Bass("TRN2", target_bir_lowering=False, debug=False)

    din = lambda name, shape: nc.dram_tensor(name, shape, f32, kind="ExternalInput")
    x_d = din("x", [N_TOK, DIM])
    c_d = din("ctx", [N_TOK, DIM])
    xr_d = din("xr", [R, DIM])
    cr_d = din("ctxr", [R, DIM])
    wqk_d = din("w_qk", [DIM, DIM])
    wcqk_d = din("w_cqk", [DIM, DIM])
    wv_d = din("w_v", [DIM, DIM])
    wcv_d = din("w_cv", [DIM, DIM])
    wout_d = din("w_out", [DIM, DIM])
    wcout_d = din("w_cout", [DIM, DIM])
    gx_d = din("g_x", [P, DIM])
    bx_d = din("b_x", [P, DIM])
    gc_d = din("g_c", [P, DIM])
    bc_d = din("b_c", [P, DIM])
    bout_d = din("bias_out", [P, DIM])
    bcout_d = din("bias_cout", [P, DIM])
    wexp_th_d = din("wexp_th", [P, HEADS, HEADS])    # [p, h, g] = W_th[g, h]
    wexp_cth_d = din("wexp_cth", [P, HEADS, HEADS])

    out_d = nc.dram_tensor("out_blk", [R, DIM], f32, kind="ExternalOutput")
    cout_d = nc.dram_tensor("cout_blk", [R, DIM], f32, kind="ExternalOutput")

    v_spill_d = nc.dram_tensor("v_spill", [8, P, DIM], f32r)  # [tok_tile, p, (h d)]

    EVICT_COPY = (nc.vector.tensor_copy if _KNOB("K_EVDVE", 0)
                  else nc.any.tensor_copy)
    _WDMA = (nc.scalar.dma_start if _KNOB("K_WDMA_ACT", 0)
             else nc.sync.dma_start)

    def mm(out, lhsT, rhs, start, stop):
        nc.tensor.matmul(out, lhsT, rhs, start=start, stop=stop)

    with tile.TileContext(nc) as tc, ExitStack() as top:
        consts = top.enter_context(tc.tile_pool(name="consts", bufs=1))
        ident = consts.tile([P, P], f32)
        make_identity(nc, ident[:])
        ones = consts.tile([P, 1], f32)
        nc.vector.memset(ones[:], 1.0)
        onesM_f = consts.tile([P, P], f32)
        nc.vector.memset(onesM_f[:], 1.0)
        onesM = consts.tile([P, P], f32r)
        nc.vector.tensor_copy(onesM[:], onesM_f[:])
        eps = consts.tile([P, 1], f32)
        nc.vector.memset(eps[:], 1e-5)

        bigp = top.enter_context(tc.tile_pool(name="big", bufs=1))
        vcvp = top.enter_context(tc.tile_pool(name="vcv", bufs=1))

        def side_pipeline(ntp, rntp, src_d, srcR_d, g_d, b_d, wT_d, wV_d,
                          v_dst):
            """Returns (dstT, dstR) big tiles for this side."""
            nT = ntp.tile([P, 8, N_TOK], f32r, tag="nt")   # [f%128, f//128, tok]
            rnT = rntp.tile([P, 8, R], f32r, tag="rnt")

            with (
                tc.tile_pool(name="ln_consts", bufs=1) as lnp,
                tc.tile_pool(name="a_work", bufs=2) as aw,
                tc.tile_pool(name="a_small", bufs=4) as asm,
                tc.tile_pool(name="a_psum", bufs=4, space="PSUM") as aps,
            ):
                g_t = lnp.tile([P, DIM], f32, tag="g")
                nc.sync.dma_start(g_t[:], g_d[:, :])
                b_t = lnp.tile([P, DIM], f32, tag="b")
                nc.sync.dma_start(b_t[:], b_d[:, :])

                def ln_transpose(src, n_tiles, dst3):
                    for it in range(n_tiles):
                        xt = aw.tile([P, DIM], f32, tag="t_a")
                        nc.sync.dma_start(xt[:], src[ts(it, P), :])
                        nmean = asm.tile([P, 1], f32, tag="nmean")
                        nc.vector.reduce_sum(
                            nmean[:], xt[:], axis=mybir.AxisListType.X
                        )
                        nc.scalar.mul(nmean[:], nmean[:], -1.0 / DIM)
                        xc = aw.tile([P, DIM], f32, tag="t_b")
                        nc.scalar.add(xc[:], xt[:], nmean[:])
                        sq = aw.tile([P, DIM], f32, tag="t_b")
                        nc.scalar.activation(
                            sq[:], xc[:], mybir.ActivationFunctionType.Square
                        )
                        var = asm.tile([P, 1], f32, tag="var")
                        nc.vector.reduce_sum(
                            var[:], sq[:], axis=mybir.AxisListType.X
                        )
                        nc.scalar.mul(var[:], var[:], 1.0 / DIM)
                        std = asm.tile([P, 1], f32, tag="std")
                        nc.scalar.activation(
                            std[:], var[:], mybir.ActivationFunctionType.Sqrt,
                            bias=eps[:],
                        )
                        rstd = asm.tile([P, 1], f32, tag="rstd")
                        nc.vector.reciprocal(rstd[:], std[:])
                        xn = aw.tile([P, DIM], f32, tag="t_a")
                        nc.vector.scalar_tensor_tensor(
                            xn[:], xc[:], rstd[:], g_t[:], op0=MULT, op1=MULT
                        )
                        nc.vector.tensor_add(xn[:], xn[:], b_t[:])
                        for ft in range(8):
                            pt = aps.tile([P, P], f32, tag="tps")
                            nc.tensor.transpose(
                                pt[:], xn[:, ts(ft, P)], ident[:]
                            )
                            EVICT_COPY(dst3[:, ft, ts(it, P)], pt[:])

                ln_transpose(src_d, 8, nT)
                ln_transpose(srcR_d, 2, rnT)

            dstT = bigp.tile([P, 8, N_TOK], f32r, tag=f"T_{wT_d.name}")
            dstR = bigp.tile([P, 8, R], f32r, tag=f"R_{wT_d.name}")

            with (
                tc.tile_pool(name="b_w", bufs=_KNOB("K_BW", 8)) as bwp,
                tc.tile_pool(name="b_wpin", bufs=1) as bwpin,
                tc.tile_pool(name="b_psum", bufs=_KNOB("K_BPS", 2), space="PSUM") as bps,
                tc.tile_pool(name="b_evict", bufs=2) as bev,
            ):
                # transposed projection: dstT[(hd), tok] = W^T @ nT
                for mt in range(8):
                    pa = bps.tile([P, 512], f32, tag="pa")
                    pb = bps.tile([P, 512], f32, tag="pb")
                    pr = bps.tile([P, R], f32, tag="pr")
                    for kt in range(8):
                        wt = bwp.tile([P, P], f32r, tag="wt")
                        _WDMA(wt[:], wT_d[ts(kt, P), ts(mt, P)].bitcast(f32r))
                        st, sp = kt == 0, kt == 7
                        mm(pa[:], wt[:], nT[:, kt, 0:512], st, sp)
                        mm(pb[:], wt[:], nT[:, kt, 512:1024], st, sp)
                        mm(pr[:], wt[:], rnT[:, kt, :], st, sp)
                    EVICT_COPY(dstT[:, mt, 0:512], pa[:])
                    EVICT_COPY(dstT[:, mt, 512:1024], pb[:])
                    EVICT_COPY(dstR[:, mt, :], pr[:])

                # natural projection: v[tok, (hd)] = nT^T @ W
                for nch in range(2):
                    wst = bwpin.tile([P, 8, 512], f32r, tag="wpin")
                    for kt in range(8):
                        nc.sync.dma_start(
                            wst[:, kt, :],
                            wV_d[ts(kt, P), ts(nch, 512)].bitcast(f32r),
                        )
                    for mt in range(8):
                        ps = bps.tile([P, 512], f32, tag="pv")
                        for kt in range(8):
                            mm(ps[:], nT[:, kt, ts(mt, P)], wst[:, kt, :],
                               kt == 0, kt == 7)
                        if v_dst[0] == "sbuf":
                            EVICT_COPY(
                                v_dst[1][:, mt, ts(nch, 512)], ps[:]
                            )
                        else:
                            ev = bev.tile([P, 512], f32r, tag="vev")
                            EVICT_COPY(ev[:], ps[:])
                            nc.sync.dma_start(
                                v_dst[1][mt, :, ts(nch, 512)], ev[:]
                            )
            return dstT, dstR

        with (
            tc.tile_pool(name="nt", bufs=1) as ntp,
            tc.tile_pool(name="rnt", bufs=1) as rntp,
        ):
            # ctx side first: cv stays resident for path 1
            cv = vcvp.tile([P, 8, DIM], f32r, tag="vcv")
            cqkT, cqR = side_pipeline(
                ntp, rntp, c_d, cr_d, gc_d, bc_d, wcqk_d, wcv_d, ("sbuf", cv)
            )
            # x side: v spilled to DRAM
            qkT, qR = side_pipeline(
                ntp, rntp, x_d, xr_d, gx_d, bx_d, wqk_d, wv_d,
                ("dram", v_spill_d)
            )

        def run_path(KT, QRt, V, wexp_d, wproj_d, bias_d, out_dram):
            with (
                tc.tile_pool(name="p_small", bufs=1) as psm,
                tc.tile_pool(name="p_U", bufs=_KNOB("K_U", 4)) as pU,
                tc.tile_pool(name="p_sigs", bufs=2) as psigs,
                tc.tile_pool(name="p_tmp", bufs=3) as ptmp,
                tc.tile_pool(name="p_acc", bufs=1) as pacc,
                tc.tile_pool(name="p_wpin", bufs=1) as pwp,
                tc.tile_pool(name="p_out", bufs=3) as pout,
                tc.tile_pool(name="p_scr", bufs=_KNOB("K_SCR", 4), space="PSUM") as pscr,
                tc.tile_pool(name="p_sig", bufs=2, space="PSUM") as psig,
                tc.tile_pool(name="p_big", bufs=_KNOB("K_BIG", 2), space="PSUM") as pbig,
            ):
                wexp = psm.tile([P, HEADS, HEADS], f32, tag="wexp")
                nc.sync.dma_start(wexp[:], wexp_d[:, :, :])
                bias = psm.tile([P, DIM], f32, tag="bias")
                nc.sync.dma_start(bias[:], bias_d[:, :])

                acc = pacc.tile([P, 2, DIM], f32, tag="acc")

                for hp in range(HEADS // 2):
                    mt_h = hp
                    # head pair (2hp, 2hp+1): K=64 matmuls at partition
                    # offsets 0 / 64 -> distinct PE row-groups, the array
                    # overlaps them (tile_position auto-derived).
                    Ua = pU.tile([P, 8, R], f32r, tag="U")
                    Ub = pU.tile([P, 8, R], f32r, tag="U")
                    Upair = [Ua, Ub]
                    for jt in range(8):
                        psSa = pscr.tile([P, R], f32, tag="scr")
                        psSb = pscr.tile([P, R], f32, tag="scr")
                        psS2 = [psSa, psSb]
                        for half in range(2):
                            off = half * DHEAD
                            mm(
                                psS2[half][:],
                                KT[off:off + DHEAD, mt_h, ts(jt, P)],
                                QRt[off:off + DHEAD, mt_h, :],
                                True, True,
                            )
                        for half in range(2):
                            nc.scalar.activation(
                                Upair[half][:, jt, :], psS2[half][:],
                                mybir.ActivationFunctionType.Exp, scale=SCALE,
                            )
                    for half in range(2):
                        U = Upair[half]
                        sigps = psig.tile([P, R], f32, tag="sigbc")
                        for jt in range(8):
                            mm(sigps[:], onesM[:], U[:, jt, :],
                               jt == 0, jt == 7)
                        siginv = psigs.tile([P, R], f32, tag="siginv")
                        nc.vector.reciprocal(siginv[:], sigps[:])
                        for jt in range(8):
                            nc.vector.tensor_mul(U[:, jt, :], U[:, jt, :],
                                                 siginv[:])

                    for h, U in ((2 * hp, Upair[0]), (2 * hp + 1, Upair[1])):
                      for m2 in range(2):
                          for nch in range(2):
                              po = pbig.tile([P, 512], f32, tag="po")
                              for jt in range(8):
                                  mm(po[:], U[:, jt, ts(m2, P)],
                                     V[:, jt, ts(nch, 512)], jt == 0, jt == 7)
                              wbc = wexp[:, h, ts(nch, 8)][:, :, None] \
                                  .to_broadcast((P, 8, DHEAD))
                              po3 = po[:].rearrange("p (g d) -> p g d", d=DHEAD)
                              if h == 0:
                                  dst = acc[:, m2, ts(nch, 512)]
                                  nc.vector.tensor_mul(
                                      dst.rearrange("p (g d) -> p g d", d=DHEAD),
                                      po3, wbc,
                                  )
                              else:
                                  tmp = ptmp.tile([P, 512], f32, tag="tmp")
                                  nc.vector.tensor_mul(
                                      tmp[:].rearrange("p (g d) -> p g d", d=DHEAD),
                                      po3, wbc,
                                  )
                                  dst = acc[:, m2, ts(nch, 512)]
                                  nc.vector.tensor_add(dst, dst, tmp[:])

                # acc^T for the final contraction
                accT = pacc.tile([P, 8, R], f32r, tag="accT")
                for m2 in range(2):
                    for gdt in range(8):
                        pt = pscr.tile([P, P], f32, tag="scr")
                        nc.tensor.transpose(
                            pt[:], acc[:, m2, ts(gdt, P)], ident[:]
                        )
                        EVICT_COPY(accT[:, gdt, ts(m2, P)], pt[:])

                # final projection + bias
                for nch in range(2):
                    wst = pwp.tile([P, 8, 512], f32r, tag="wpin")
                    for kt in range(8):
                        nc.sync.dma_start(
                            wst[:, kt, :],
                            wproj_d[ts(kt, P), ts(nch, 512)].bitcast(f32r),
                        )
                    for m2 in range(2):
                        pf = pbig.tile([P, 512], f32, tag="po")
                        for gdt in range(8):
                            mm(pf[:], accT[:, gdt, ts(m2, P)], wst[:, gdt, :],
                               gdt == 0, gdt == 7)
                        ot = pout.tile([P, 512], f32, tag="ot")
                        nc.vector.scalar_tensor_tensor(
                            ot[:], pf[:], 1.0, bias[:, ts(nch, 512)],
                            op0=MULT, op1=ADD,
                        )
                        nc.sync.dma_start(
                            out_dram[ts(m2, P), ts(nch, 512)], ot[:]
                        )

        # path 1: out rows (x queries attend to context; values = cv)
        run_path(cqkT, qR, cv, wexp_th_d, wout_d, bout_d, out_d)

        # reload spilled v into cv's slot (tag-shared, WAR handled by Tile)
        v2 = vcvp.tile([P, 8, DIM], f32r, tag="vcv")
        for mt in range(8):
            nc.sync.dma_start(v2[:, mt, :], v_spill_d[mt, :, :])
        # path 2: cout rows (context queries attend to x; values = v)
        run_path(qkT, cqR, v2, wexp_cth_d, wcout_d, bcout_d, cout_d)

    return nc


def _prep_in_maps(inputs):
    g = lambda k: np.ascontiguousarray(np.asarray(inputs[k], dtype=np.float32))
    x = g("x")
    ctx = g("context")
    bcast = lambda v: np.ascontiguousarray(
        np.broadcast_to(np.asarray(v, np.float32), (P, DIM))
    )
    common = {
        "w_qk": g("W_qk"), "w_cqk": g("W_cqk"),
        "w_v": g("W_v"), "w_cv": g("W_cv"),
        "w_out": g("W_out"), "w_cout": g("W_cout"),
        "g_x": bcast(inputs["ln_g"]), "b_x": bcast(inputs["ln_b"]),
        "g_c": bcast(inputs["cln_g"]), "b_c": bcast(inputs["cln_b"]),
        "bias_out": bcast(inputs["b_out"]), "bias_cout": bcast(inputs["b_cout"]),
        "wexp_th": np.ascontiguousarray(
            np.broadcast_to(g("W_th").T[None, :, :], (P, HEADS, HEADS))
        ),
        "wexp_cth": np.ascontiguousarray(
            np.broadcast_to(g("W_cth").T[None, :, :], (P, HEADS, HEADS))
        ),
    }
    in_maps = []
    for c in range(NCORES):
        b, r0 = c // 4, (c % 4) * R
        in_maps.append({
            "x": np.ascontiguousarray(x[b]),
            "ctx": np.ascontiguousarray(ctx[b]),
            "xr": np.ascontiguousarray(x[b, r0:r0 + R]),
            "ctxr": np.ascontiguousarray(ctx[b, r0:r0 + R]),
            **common,
        })
    return in_maps


def kernel(**inputs):
    from concourse.bass_utils import run_bass_kernel_spmd

    if "nc" not in _CACHE:
        _CACHE["nc"] = build_program()
    nc = _CACHE["nc"]

    in_maps = _prep_in_maps(inputs)
    res = run_bass_kernel_spmd(nc, in_maps, core_ids=list(range(NCORES)))

    out = np.empty((2, N_TOK, DIM), np.float32)
    cout = np.empty((2, N_TOK, DIM), np.float32)
    for c in range(NCORES):
        b, r0 = c // 4, (c % 4) * R
        out[b, r0:r0 + R] = res.results[c]["out_blk"]
        cout[b, r0:r0 + R] = res.results[c]["cout_blk"]
    return out, cout



# revision 1
# speedup vs baseline: 1.3382x; 1.3382x over previous
"""Bidirectional cross-attention with talking heads — TRN2 Bass kernel.

Sharding: 8 cores = 2 batches x 4 row-blocks of 256 (data parallel over
batch and sequence block). Each core computes, for its (batch, block):
  - out rows  [256, 1024]  (x attends to context, row-softmax path)
  - cout rows [256, 1024]  (context attends to x, col-softmax path)
No collectives; the host assembles the 8 blocks.

Per-core schedule (SBUF-budgeted):
  ctx side: LN -> PE-transpose (feat on partitions) -> cqkT/cqR projection
            -> cv natural projection (kept in SBUF)
  x side:   same -> qkT/qR -> v natural projection (spilled to DRAM)
  path 1 (out rows), reload v, path 2 (cout rows).

Per-path:
  S^T[h]   = matmul(KT_h, QR_h)            [keypos on partitions, qpos free]
  U[h]     = exp(S^T * scale)              (ACT, PSUM->SBUF)
  sigma    = U^T @ ones                    (PE) softmax denominators
  out_h    = U[h].T @ V                    (PE, fp32r, PSUM)
  evict    = (out_h * sigma^-1) * W_th[g,h]  (DVE scalar_tensor_tensor),
             accumulated over h in SBUF  == talking-heads mix fused in
  final    = transpose(acc) @ W_out + b_out  (PE + DVE)

Masks are structurally all-ones for this problem (spec fill=ones); the
reference's mask application is then the identity, so they are ignored.
exp() without max-subtraction is safe: |S*scale| <~ 7.
"""

import os
import numpy as np
from contextlib import ExitStack

_KNOB = lambda k, d: int(os.environ.get(k, d))

P = 128
N_TOK = 1024
DIM = 1024
HEADS = 16
DHEAD = 64
R = 256
SCALE = DHEAD ** -0.5
NCORES = 8

_CACHE = {}


def _patch_tile_drain(tile, mybir):
    """This container's walrus rejects >1 sync wait on an InstDrain
    ("Too many sync wait commands"). Split the TileContext tail drain's
    waits across a chain of single-wait drains on the same engine."""
    if getattr(tile.TileContext, "_drain_split_patched", False):
        return

    def _drain_and_barrier(self, tick_clock, wait_clock):
        drain_inst = self.nc.sync.drain()
        wait_clock.add_sem_waits(
            drain_inst.ins, tile.ScopedClock({None: tick_clock.global_clock})
        )
        si = drain_inst.ins.sync_info
        waits = list(si.on_wait) if si is not None else []
        if len(waits) > 1:
            drain_inst.ins.sync_info = mybir.SyncInfo(
                on_wait=[waits[0]], on_update=list(si.on_update)
            )
            for w in waits[1:]:
                extra = self.nc.sync.drain()
                extra.ins.sync_info = mybir.SyncInfo(on_wait=[w], on_update=[])

        self.nc.all_engine_barrier()
        assert self.sems is not None
        popped = self.nc._tile_sem_poison_stack.pop()
        assert popped is self._sem_poison
        self.nc.clear_and_free_semaphores(list(self.sems.allocated().values()))
        self.nc.all_engine_barrier()

    tile.TileContext._drain_and_barrier = _drain_and_barrier
    tile.TileContext._drain_split_patched = True


_WSPLIT_MAX = 1  # max sync waits this walrus accepts per instruction


def _patch_tile_wait_split(tile, mybir):
    """Split instructions carrying more than _WSPLIT_MAX sem-waits: move the
    excess onto same-engine NoOps committed immediately before (same basic
    block, so engine program order preserves the wait semantics)."""
    if getattr(tile.TileContext, "_wait_split_patched", False):
        return
    orig = tile.TileContext._commit_and_lower
    counter = [0]

    def _commit_and_lower(self, inst, *args, **kwargs):
        si = getattr(inst, "sync_info", None)
        eng = getattr(inst, "engine", None)
        if si is not None and eng is not None and len(si.on_wait) > _WSPLIT_MAX:
            waits = list(si.on_wait)
            keep = waits[-_WSPLIT_MAX:]
            for w in waits[:-_WSPLIT_MAX]:
                counter[0] += 1
                nop = mybir.InstNoOp(
                    name=f"I-wsplit-{counter[0]}",
                    engine=eng, ins=[], outs=[],
                    sync_info=mybir.SyncInfo(on_wait=[w], on_update=[]),
                )
                self._add_instruction(nop)
            inst.sync_info = mybir.SyncInfo(
                on_wait=keep, on_update=list(si.on_update)
            )
        return orig(self, inst, *args, **kwargs)

    tile.TileContext._commit_and_lower = _commit_and_lower
    tile.TileContext._wait_split_patched = True


def build_program():
    import concourse.bass as bass
    import concourse.mybir as mybir
    import concourse.tile as tile
    from concourse.masks import make_identity

    _patch_tile_drain(tile, mybir)
    _patch_tile_wait_split(tile, mybir)

    f32 = mybir.dt.float32
    f32r = mybir.dt.float32r
    ts = bass.ts
    MULT = mybir.AluOpType.mult
    ADD = mybir.AluOpType.add

    nc = bass.Bass("TRN2", target_bir_lowering=False, debug=False)

    din = lambda name, shape: nc.dram_tensor(name, shape, f32, kind="ExternalInput")
    x_d = din("x", [N_TOK, DIM])
    c_d = din("ctx", [N_TOK, DIM])
    xr_d = din("xr", [R, DIM])
    cr_d = din("ctxr", [R, DIM])
    wqk_d = din("w_qk", [DIM, DIM])
    wcqk_d = din("w_cqk", [DIM, DIM])
    wv_d = din("w_v", [DIM, DIM])
    wcv_d = din("w_cv", [DIM, DIM])
    wout_d = din("w_out", [DIM, DIM])
    wcout_d = din("w_cout", [DIM, DIM])
    gx_d = din("g_x", [P, DIM])
    bx_d = din("b_x", [P, DIM])
    gc_d = din("g_c", [P, DIM])
    bc_d = din("b_c", [P, DIM])
    bout_d = din("bias_out", [P, DIM])
    bcout_d = din("bias_cout", [P, DIM])
    wexp_th_d = din("wexp_th", [P, HEADS, HEADS])    # [p, h, g] = W_th[g, h]
    wexp_cth_d = din("wexp_cth", [P, HEADS, HEADS])

    out_d = nc.dram_tensor("out_blk", [R, DIM], f32, kind="ExternalOutput")
    cout_d = nc.dram_tensor("cout_blk", [R, DIM], f32, kind="ExternalOutput")

    v_spill_d = nc.dram_tensor("v_spill", [8, P, DIM], f32r)  # [tok_tile, p, (h d)]

    EVICT_COPY = (nc.vector.tensor_copy if _KNOB("K_EVDVE", 0)
                  else nc.any.tensor_copy)
    _WDMA = (nc.scalar.dma_start if _KNOB("K_WDMA_ACT", 0)
             else nc.sync.dma_start)

    def mm(out, lhsT, rhs, start, stop):
        nc.tensor.matmul(out, lhsT, rhs, start=start, stop=stop)

    with tile.TileContext(nc) as tc, ExitStack() as top:
        consts = top.enter_context(tc.tile_pool(name="consts", bufs=1))
        ident = consts.tile([P, P], f32)
        make_identity(nc, ident[:])
        ones = consts.tile([P, 1], f32)
        nc.vector.memset(ones[:], 1.0)
        onesM_f = consts.tile([P, P], f32)
        nc.vector.memset(onesM_f[:], 1.0)
        onesM = consts.tile([P, P], f32r)
        nc.vector.tensor_copy(onesM[:], onesM_f[:])
        eps = consts.tile([P, 1], f32)
        nc.vector.memset(eps[:], 1e-5)

        bigp = top.enter_context(tc.tile_pool(name="big", bufs=1))
        vcvp = top.enter_context(tc.tile_pool(name="vcv", bufs=1))

        def side_pipeline(ntp, rntp, src_d, srcR_d, g_d, b_d, wT_d, wV_d,
                          v_dst):
            """Returns (dstT, dstR) big tiles for this side."""
            nT = ntp.tile([P, 8, N_TOK], f32r, tag="nt")   # [f%128, f//128, tok]
            rnT = rntp.tile([P, 8, R], f32r, tag="rnt")

            with (
                tc.tile_pool(name="ln_consts", bufs=1) as lnp,
                tc.tile_pool(name="a_work", bufs=2) as aw,
                tc.tile_pool(name="a_small", bufs=4) as asm,
                tc.tile_pool(name="a_psum", bufs=4, space="PSUM") as aps,
            ):
                g_t = lnp.tile([P, DIM], f32, tag="g")
                nc.sync.dma_start(g_t[:], g_d[:, :])
                b_t = lnp.tile([P, DIM], f32, tag="b")
                nc.sync.dma_start(b_t[:], b_d[:, :])

                def ln_transpose(src, n_tiles, dst3):
                    for it in range(n_tiles):
                        xt = aw.tile([P, DIM], f32, tag="t_a")
                        nc.sync.dma_start(xt[:], src[ts(it, P), :])
                        nmean = asm.tile([P, 1], f32, tag="nmean")
                        nc.vector.reduce_sum(
                            nmean[:], xt[:], axis=mybir.AxisListType.X
                        )
                        nc.scalar.mul(nmean[:], nmean[:], -1.0 / DIM)
                        xc = aw.tile([P, DIM], f32, tag="t_b")
                        nc.scalar.add(xc[:], xt[:], nmean[:])
                        sq = aw.tile([P, DIM], f32, tag="t_b")
                        nc.scalar.activation(
                            sq[:], xc[:], mybir.ActivationFunctionType.Square
                        )
                        var = asm.tile([P, 1], f32, tag="var")
                        nc.vector.reduce_sum(
                            var[:], sq[:], axis=mybir.AxisListType.X
                        )
                        nc.scalar.mul(var[:], var[:], 1.0 / DIM)
                        std = asm.tile([P, 1], f32, tag="std")
                        nc.scalar.activation(
                            std[:], var[:], mybir.ActivationFunctionType.Sqrt,
                            bias=eps[:],
                        )
                        rstd = asm.tile([P, 1], f32, tag="rstd")
                        nc.vector.reciprocal(rstd[:], std[:])
                        xn = aw.tile([P, DIM], f32, tag="t_a")
                        nc.vector.scalar_tensor_tensor(
                            xn[:], xc[:], rstd[:], g_t[:], op0=MULT, op1=MULT
                        )
                        nc.vector.tensor_add(xn[:], xn[:], b_t[:])
                        for ft in range(8):
                            pt = aps.tile([P, P], f32, tag="tps")
                            nc.tensor.transpose(
                                pt[:], xn[:, ts(ft, P)], ident[:]
                            )
                            EVICT_COPY(dst3[:, ft, ts(it, P)], pt[:])

                ln_transpose(src_d, 8, nT)
                ln_transpose(srcR_d, 2, rnT)

            dstT = bigp.tile([P, 8, N_TOK], f32r, tag=f"T_{wT_d.name}")
            dstR = bigp.tile([P, 8, R], f32r, tag=f"R_{wT_d.name}")

            with (
                tc.tile_pool(name="b_w", bufs=_KNOB("K_BW", 8)) as bwp,
                tc.tile_pool(name="b_wpin", bufs=1) as bwpin,
                tc.tile_pool(name="b_psum", bufs=_KNOB("K_BPS", 2), space="PSUM") as bps,
                tc.tile_pool(name="b_evict", bufs=2) as bev,
            ):
                # transposed projection: dstT[(hd), tok] = W^T @ nT
                for mt in range(8):
                    pa = bps.tile([P, 512], f32, tag="pa")
                    pb = bps.tile([P, 512], f32, tag="pb")
                    pr = bps.tile([P, R], f32, tag="pr")
                    for kt in range(8):
                        wt = bwp.tile([P, P], f32r, tag="wt")
                        _WDMA(wt[:], wT_d[ts(kt, P), ts(mt, P)].bitcast(f32r))
                        st, sp = kt == 0, kt == 7
                        mm(pa[:], wt[:], nT[:, kt, 0:512], st, sp)
                        mm(pb[:], wt[:], nT[:, kt, 512:1024], st, sp)
                        mm(pr[:], wt[:], rnT[:, kt, :], st, sp)
                    EVICT_COPY(dstT[:, mt, 0:512], pa[:])
                    EVICT_COPY(dstT[:, mt, 512:1024], pb[:])
                    EVICT_COPY(dstR[:, mt, :], pr[:])

                # natural projection: v[tok, (hd)] = nT^T @ W
                for nch in range(2):
                    wst = bwpin.tile([P, 8, 512], f32r, tag="wpin")
                    for kt in range(8):
                        nc.sync.dma_start(
                            wst[:, kt, :],
                            wV_d[ts(kt, P), ts(nch, 512)].bitcast(f32r),
                        )
                    for mt in range(8):
                        ps = bps.tile([P, 512], f32, tag="pv")
                        for kt in range(8):
                            mm(ps[:], nT[:, kt, ts(mt, P)], wst[:, kt, :],
                               kt == 0, kt == 7)
                        if v_dst[0] == "sbuf":
                            EVICT_COPY(
                                v_dst[1][:, mt, ts(nch, 512)], ps[:]
                            )
                        else:
                            ev = bev.tile([P, 512], f32r, tag="vev")
                            EVICT_COPY(ev[:], ps[:])
                            nc.sync.dma_start(
                                v_dst[1][mt, :, ts(nch, 512)], ev[:]
                            )
            return dstT, dstR

        with (
            tc.tile_pool(name="nt", bufs=1) as ntp,
            tc.tile_pool(name="rnt", bufs=1) as rntp,
        ):
            # ctx side first: cv stays resident for path 1
            cv = vcvp.tile([P, 8, DIM], f32r, tag="vcv")
            cqkT, cqR = side_pipeline(
                ntp, rntp, c_d, cr_d, gc_d, bc_d, wcqk_d, wcv_d, ("sbuf", cv)
            )
            # x side: v spilled to DRAM
            qkT, qR = side_pipeline(
                ntp, rntp, x_d, xr_d, gx_d, bx_d, wqk_d, wv_d,
                ("dram", v_spill_d)
            )

        def run_path(KT, QRt, V, wexp_d, wproj_d, bias_d, out_dram):
            with (
                tc.tile_pool(name="p_small", bufs=1) as psm,
                tc.tile_pool(name="p_U", bufs=_KNOB("K_U", 4)) as pU,
                tc.tile_pool(name="p_sigs", bufs=2) as psigs,
                tc.tile_pool(name="p_tmp", bufs=3) as ptmp,
                tc.tile_pool(name="p_acc", bufs=1) as pacc,
                tc.tile_pool(name="p_wpin", bufs=1) as pwp,
                tc.tile_pool(name="p_out", bufs=3) as pout,
                tc.tile_pool(name="p_scr", bufs=_KNOB("K_SCR", 4), space="PSUM") as pscr,
                tc.tile_pool(name="p_sig", bufs=2, space="PSUM") as psig,
                tc.tile_pool(name="p_big", bufs=_KNOB("K_BIG", 2), space="PSUM") as pbig,
            ):
                wexp = psm.tile([P, HEADS, HEADS], f32, tag="wexp")
                nc.sync.dma_start(wexp[:], wexp_d[:, :, :])
                bias = psm.tile([P, DIM], f32, tag="bias")
                nc.sync.dma_start(bias[:], bias_d[:, :])

                acc = pacc.tile([P, 2, DIM], f32, tag="acc")

                for hp in range(HEADS // 2):
                    mt_h = hp
                    # head pair (2hp, 2hp+1): K=64 matmuls at partition
                    # offsets 0 / 64 -> distinct PE row-groups, the array
                    # overlaps them (tile_position auto-derived).
                    Ua = pU.tile([P, 8, R], f32r, tag="U")
                    Ub = pU.tile([P, 8, R], f32r, tag="U")
                    Upair = [Ua, Ub]
                    for jt in range(8):
                        psSa = pscr.tile([P, R], f32, tag="scr")
                        psSb = pscr.tile([P, R], f32, tag="scr")
                        psS2 = [psSa, psSb]
                        for half in range(2):
                            off = half * DHEAD
                            mm(
                                psS2[half][:],
                                KT[off:off + DHEAD, mt_h, ts(jt, P)],
                                QRt[off:off + DHEAD, mt_h, :],
                                True, True,
                            )
                        for half in range(2):
                            nc.scalar.activation(
                                Upair[half][:, jt, :], psS2[half][:],
                                mybir.ActivationFunctionType.Exp, scale=SCALE,
                            )
                    for half in range(2):
                        U = Upair[half]
                        sigps = psig.tile([P, R], f32, tag="sigbc")
                        for jt in range(8):
                            mm(sigps[:], onesM[:], U[:, jt, :],
                               jt == 0, jt == 7)
                        siginv = psigs.tile([P, R], f32, tag="siginv")
                        nc.vector.reciprocal(siginv[:], sigps[:])
                        for jt in range(8):
                            nc.vector.tensor_mul(U[:, jt, :], U[:, jt, :],
                                                 siginv[:])

                    for h, U in ((2 * hp, Upair[0]), (2 * hp + 1, Upair[1])):
                      for m2 in range(2):
                          for nch in range(2):
                              po = pbig.tile([P, 512], f32, tag="po")
                              for jt in range(8):
                                  mm(po[:], U[:, jt, ts(m2, P)],
                                     V[:, jt, ts(nch, 512)], jt == 0, jt == 7)
                              wbc = wexp[:, h, ts(nch, 8)][:, :, None] \
                                  .to_broadcast((P, 8, DHEAD))
                              po3 = po[:].rearrange("p (g d) -> p g d", d=DHEAD)
                              if h == 0:
                                  dst = acc[:, m2, ts(nch, 512)]
                                  nc.vector.tensor_mul(
                                      dst.rearrange("p (g d) -> p g d", d=DHEAD),
                                      po3, wbc,
                                  )
                              else:
                                  tmp = ptmp.tile([P, 512], f32, tag="tmp")
                                  nc.vector.tensor_mul(
                                      tmp[:].rearrange("p (g d) -> p g d", d=DHEAD),
                                      po3, wbc,
                                  )
                                  dst = acc[:, m2, ts(nch, 512)]
                                  nc.vector.tensor_add(dst, dst, tmp[:])

                # acc^T for the final contraction
                accT = pacc.tile([P, 8, R], f32r, tag="accT")
                for m2 in range(2):
                    for gdt in range(8):
                        pt = pscr.tile([P, P], f32, tag="scr")
                        nc.tensor.transpose(
                            pt[:], acc[:, m2, ts(gdt, P)], ident[:]
                        )
                        EVICT_COPY(accT[:, gdt, ts(m2, P)], pt[:])

                # final projection + bias
                for nch in range(2):
                    wst = pwp.tile([P, 8, 512], f32r, tag="wpin")
                    for kt in range(8):
                        nc.sync.dma_start(
                            wst[:, kt, :],
                            wproj_d[ts(kt, P), ts(nch, 512)].bitcast(f32r),
                        )
                    for m2 in range(2):
                        pf = pbig.tile([P, 512], f32, tag="po")
                        for gdt in range(8):
                            mm(pf[:], accT[:, gdt, ts(m2, P)], wst[:, gdt, :],
                               gdt == 0, gdt == 7)
                        ot = pout.tile([P, 512], f32, tag="ot")
                        nc.vector.scalar_tensor_tensor(
                            ot[:], pf[:], 1.0, bias[:, ts(nch, 512)],
                            op0=MULT, op1=ADD,
                        )
                        nc.sync.dma_start(
                            out_dram[ts(m2, P), ts(nch, 512)], ot[:]
                        )

        # path 1: out rows (x queries attend to context; values = cv)
        run_path(cqkT, qR, cv, wexp_th_d, wout_d, bout_d, out_d)

        # reload spilled v into cv's slot (tag-shared, WAR handled by Tile)
        v2 = vcvp.tile([P, 8, DIM], f32r, tag="vcv")
        for mt in range(8):
            nc.sync.dma_start(v2[:, mt, :], v_spill_d[mt, :, :])
        # path 2: cout rows (context queries attend to x; values = v)
        run_path(qkT, cqR, v2, wexp_cth_d, wcout_d, bcout_d, cout_d)

    return nc


def _prep_in_maps(inputs):
    g = lambda k: np.ascontiguousarray(np.asarray(inputs[k], dtype=np.float32))
    x = g("x")
    ctx = g("context")
    bcast = lambda v: np.ascontiguousarray(
        np.broadcast_to(np.asarray(v, np.float32), (P, DIM))
    )
    common = {
        "w_qk": g("W_qk"), "w_cqk": g("W_cqk"),
        "w_v": g("W_v"), "w_cv": g("W_cv"),
        "w_out": g("W_out"), "w_cout": g("W_cout"),
        "g_x": bcast(inputs["ln_g"]), "b_x": bcast(inputs["ln_b"]),
        "g_c": bcast(inputs["cln_g"]), "b_c": bcast(inputs["cln_b"]),
        "bias_out": bcast(inputs["b_out"]), "bias_cout": bcast(inputs["b_cout"]),
        "wexp_th": np.ascontiguousarray(
            np.broadcast_to(g("W_th").T[None, :, :], (P, HEADS, HEADS))
        ),
        "wexp_cth": np.ascontiguousarray(
            np.broadcast_to(g("W_cth").T[None, :, :], (P, HEADS, HEADS))
        ),
    }
    in_maps = []
    for c in range(NCORES):
        b, r0 = c // 4, (c % 4) * R
        in_maps.append({
            "x": np.ascontiguousarray(x[b]),
            "ctx": np.ascontiguousarray(ctx[b]),
            "xr": np.ascontiguousarray(x[b, r0:r0 + R]),
            "ctxr": np.ascontiguousarray(ctx[b, r0:r0 + R]),
            **common,
        })
    return in_maps


def kernel(**inputs):
    from concourse.bass_utils import run_bass_kernel_spmd

    if "nc" not in _CACHE:
        _CACHE["nc"] = build_program()
    nc = _CACHE["nc"]

    in_maps = _prep_in_maps(inputs)
    res = run_bass_kernel_spmd(nc, in_maps, core_ids=list(range(NCORES)))

    out = np.empty((2, N_TOK, DIM), np.float32)
    cout = np.empty((2, N_TOK, DIM), np.float32)
    for c in range(NCORES):
        b, r0 = c // 4, (c % 4) * R
        out[b, r0:r0 + R] = res.results[c]["out_blk"]
        cout[b, r0:r0 + R] = res.results[c]["cout_blk"]
    return out, cout

